# revision 8
# baseline (speedup 1.0000x reference)
"""Trainium2 Bass kernel for nn_CRA_46797963657479.

Math: the reference builds per-batch gram matrix A = cat_phi^T cat_phi
([B,392,392]) and feeds concat(A, A^T) through big 1x1 convs.  Since A is
symmetric and everything after cat_phi is linear, the whole tail collapses:

    W[b, l] = (u3 + cat_phi[b] @ u4) . cat_phi[b][:, l] + K
    out[b]  = xp[b] * W[b, :N] + yp[b] * W[b, N:]

with u3 = w5a @ w3, u4 = w5b @ (w4[:, :2N] + w4[:, 2N:]),
K = w5a.b3 + w5b.b4 + b5.  BN folds into the conv weights.  What remains per
batch is two 192x192 matmuls (phi_x, phi_y), a weighted free-dim reduction
(z), one more matmul for W, and an elementwise combine -> memory-bound.

Sharding: pure data parallel, batch 256 -> 32 per core on 8 cores.
"""

import os
import numpy as np

import concourse.bass as bass
import concourse.bacc as bacc
import concourse.tile as tile
from concourse import mybir
from concourse.bass_utils import run_bass_kernel_spmd

F32 = mybir.dt.float32
F32R = mybir.dt.float32r

B, N, C = 256, 196, 192
NCORES = 8
NB = B // NCORES          # 32 batches per core
NPAIR = NB // 2           # 16 pairs per core
L = 2 * N                 # 392 free columns per pair tile / per stream-pack
CLO, CHI = 128, C - 128   # 128 + 64 channel split
CHIA = CHI + 1            # hi chunk augmented with a ones-row (folds +K)

_CACHE = {}


def _build_program():
    nc = bacc.Bacc("TRN2", target_bir_lowering=False, debug=False)

    xy = nc.dram_tensor("xy", [NB, 2, C, N], F32R, kind="ExternalInput")
    out = nc.dram_tensor("out", [NB, C, N], F32, kind="ExternalOutput")

    # weight tiles (lhsT = W^T chunks; m-dim of the hi tiles has a zero
    # 65th column so ACT bias=1 writes a ones-row into phi_hi)
    wxa = nc.dram_tensor("wxa", [CLO, CLO], F32R, kind="ExternalInput")
    wxb = nc.dram_tensor("wxb", [CLO, CHIA], F32R, kind="ExternalInput")
    wxc = nc.dram_tensor("wxc", [CHI, CLO], F32R, kind="ExternalInput")
    wxd = nc.dram_tensor("wxd", [CHI, CHIA], F32R, kind="ExternalInput")
    wya = nc.dram_tensor("wya", [CLO, CLO], F32R, kind="ExternalInput")
    wyb = nc.dram_tensor("wyb", [CLO, CHIA], F32R, kind="ExternalInput")
    wyc = nc.dram_tensor("wyc", [CHI, CLO], F32R, kind="ExternalInput")
    wyd = nc.dram_tensor("wyd", [CHI, CHIA], F32R, kind="ExternalInput")
    c1lo = nc.dram_tensor("c1lo", [CLO, 1], F32, kind="ExternalInput")
    c1hi = nc.dram_tensor("c1hi", [CHIA, 1], F32, kind="ExternalInput")
    c2lo = nc.dram_tensor("c2lo", [CLO, 1], F32, kind="ExternalInput")
    c2hi = nc.dram_tensor("c2hi", [CHIA, 1], F32, kind="ExternalInput")
    u4lo = nc.dram_tensor("u4lo", [CLO, L], F32, kind="ExternalInput")
    u4hi = nc.dram_tensor("u4hi", [CHIA, L], F32, kind="ExternalInput")
    u3lo = nc.dram_tensor("u3lo", [CLO, 1], F32, kind="ExternalInput")
    u3hi = nc.dram_tensor("u3hi", [CHIA, 1], F32, kind="ExternalInput")
    onesd = nc.dram_tensor("onesd", [CLO, CLO], F32, kind="ExternalInput")

    xyv = xy.rearrange("b s c n -> s c b n")     # [2, C, NB, N]
    outv = out.rearrange("b c n -> c b n")       # [C, NB, N]

    with tile.TileContext(nc) as tc:
        with (
            tc.tile_pool(name="consts", bufs=1) as consts,
            tc.tile_pool(name="xin", bufs=3) as xin,
            tc.tile_pool(name="phi", bufs=2) as phip,
            tc.tile_pool(name="junk", bufs=2) as junkp,
            tc.tile_pool(name="qp", bufs=3) as qp,
            tc.tile_pool(name="work", bufs=2) as work,
            tc.tile_pool(name="outp", bufs=3) as outp,
            tc.tile_pool(name="psph", bufs=1, space="PSUM") as psph,
            tc.tile_pool(name="psw", bufs=1, space="PSUM") as psw,
        ):
            def cload(dram, shape, dt=F32):
                t = consts.tile(shape, dt, tag=dram.name)
                nc.sync.dma_start(out=t[:], in_=dram[:])
                return t

            twxa = cload(wxa, [CLO, CLO], F32R)
            twxb = cload(wxb, [CLO, CHIA], F32R)
            twxc = cload(wxc, [CHI, CLO], F32R)
            twxd = cload(wxd, [CHI, CHIA], F32R)
            twya = cload(wya, [CLO, CLO], F32R)
            twyb = cload(wyb, [CLO, CHIA], F32R)
            twyc = cload(wyc, [CHI, CLO], F32R)
            twyd = cload(wyd, [CHI, CHIA], F32R)
            tc1lo = cload(c1lo, [CLO, 1])
            tc1hi = cload(c1hi, [CHIA, 1])
            tc2lo = cload(c2lo, [CLO, 1])
            tc2hi = cload(c2hi, [CHIA, 1])
            tu4lo = cload(u4lo, [CLO, L])
            tu4hi = cload(u4hi, [CHIA, L])
            tu3lo = cload(u3lo, [CLO, 1])
            tu3hi = cload(u3hi, [CHIA, 1])
            tones = cload(onesd, [CLO, CLO])

            def f(ap):
                return ap.bitcast(F32)

            relu = mybir.ActivationFunctionType.Relu
            mult = mybir.AluOpType.mult
            add = mybir.AluOpType.add
            byp = mybir.AluOpType.bypass

            for t in range(NPAIR):
                b0 = 2 * t
                # ---- loads: [channels, 2 batches, N] ----
                xlo = xin.tile([CLO, 2, N], F32R)
                xhi = xin.tile([CHI, 2, N], F32R)
                ylo = xin.tile([CLO, 2, N], F32R)
                yhi = xin.tile([CHI, 2, N], F32R)
                nc.sync.dma_start(out=xlo[:], in_=xyv[0, 0:CLO, b0:b0 + 2, :])
                nc.sync.dma_start(out=xhi[:], in_=xyv[0, CLO:C, b0:b0 + 2, :])
                nc.sync.dma_start(out=ylo[:], in_=xyv[1, 0:CLO, b0:b0 + 2, :])
                nc.sync.dma_start(out=yhi[:], in_=xyv[1, CLO:C, b0:b0 + 2, :])
                xlo2 = xlo[:].rearrange("p a n -> p (a n)")
                xhi2 = xhi[:].rearrange("p a n -> p (a n)")
                ylo2 = ylo[:].rearrange("p a n -> p (a n)")
                yhi2 = yhi[:].rearrange("p a n -> p (a n)")

                # ---- phi matmuls (pair-packed, 392 moving cols) ----
                ps_xlo = psph.tile([CLO, L], F32)
                ps_xhi = psph.tile([CHIA, L], F32)
                ps_ylo = psph.tile([CLO, L], F32)
                ps_yhi = psph.tile([CHIA, L], F32)
                nc.tensor.matmul(ps_xlo[:], twxa[:], xlo2, start=True, stop=False)
                nc.tensor.matmul(ps_xlo[:], twxc[:], xhi2, start=False, stop=True)
                nc.tensor.matmul(ps_xhi[:], twxb[:], xlo2, start=True, stop=False)
                nc.tensor.matmul(ps_xhi[:], twxd[:], xhi2, start=False, stop=True)
                nc.tensor.matmul(ps_ylo[:], twya[:], ylo2, start=True, stop=False)
                nc.tensor.matmul(ps_ylo[:], twyc[:], yhi2, start=False, stop=True)
                nc.tensor.matmul(ps_yhi[:], twyb[:], ylo2, start=True, stop=False)
                nc.tensor.matmul(ps_yhi[:], twyd[:], yhi2, start=False, stop=True)

                # ---- relu+bias, repacked per batch: [phi_x(b) | phi_y(b)] ----
                phiA_lo = phip.tile([CLO, L], F32R)
                phiB_lo = phip.tile([CLO, L], F32R)
                phiA_hi = phip.tile([CHIA, L], F32R)
                phiB_hi = phip.tile([CHIA, L], F32R)
                nc.scalar.activation(phiA_lo[:, 0:N], ps_xlo[:, 0:N], relu, bias=tc1lo[:])
                nc.scalar.activation(phiA_lo[:, N:L], ps_ylo[:, 0:N], relu, bias=tc2lo[:])
                nc.scalar.activation(phiB_lo[:, 0:N], ps_xlo[:, N:L], relu, bias=tc1lo[:])
                nc.scalar.activation(phiB_lo[:, N:L], ps_ylo[:, N:L], relu, bias=tc2lo[:])
                nc.scalar.activation(phiA_hi[:, 0:N], ps_xhi[:, 0:N], relu, bias=tc1hi[:])
                nc.scalar.activation(phiA_hi[:, N:L], ps_yhi[:, 0:N], relu, bias=tc2hi[:])
                nc.scalar.activation(phiB_hi[:, 0:N], ps_xhi[:, N:L], relu, bias=tc1hi[:])
                nc.scalar.activation(phiB_hi[:, N:L], ps_yhi[:, N:L], relu, bias=tc2hi[:])

                # ---- z then q = u3 + phi @ u4 (fused mult+reduce, u3 as init) ----
                jA_lo = junkp.tile([CLO, L], F32, tag="j_lo")
                jB_lo = junkp.tile([CLO, L], F32, tag="j_lo")
                jA_hi = junkp.tile([CHIA, L], F32, tag="j_hi")
                jB_hi = junkp.tile([CHIA, L], F32, tag="j_hi")
                zA_lo = qp.tile([CLO, 1], F32)
                zB_lo = qp.tile([CLO, 1], F32)
                zA_hi = qp.tile([CHIA, 1], F32)
                zB_hi = qp.tile([CHIA, 1], F32)
                nc.vector.scalar_tensor_tensor(
                    out=jA_lo[:], in0=f(phiA_lo[:]), scalar=1.0, in1=tu4lo[:],
                    op0=byp, op1=mult, accum_out=zA_lo[:])
                nc.vector.scalar_tensor_tensor(
                    out=jB_lo[:], in0=f(phiB_lo[:]), scalar=1.0, in1=tu4lo[:],
                    op0=byp, op1=mult, accum_out=zB_lo[:])
                nc.vector.scalar_tensor_tensor(
                    out=jA_hi[:], in0=f(phiA_hi[:]), scalar=1.0, in1=tu4hi[:],
                    op0=byp, op1=mult, accum_out=zA_hi[:])
                nc.vector.scalar_tensor_tensor(
                    out=jB_hi[:], in0=f(phiB_hi[:]), scalar=1.0, in1=tu4hi[:],
                    op0=byp, op1=mult, accum_out=zB_hi[:])

                # ---- Q = ones*(z) + u3 broadcast along 128 lanes (W stationary) ----
                QA_lo = qp.tile([CLO, CLO], F32R, tag="Q_lo")
                QB_lo = qp.tile([CLO, CLO], F32R, tag="Q_lo")
                QA_hi = qp.tile([CHIA, CLO], F32R, tag="Q_hi")
                QB_hi = qp.tile([CHIA, CLO], F32R, tag="Q_hi")
                nc.vector.tensor_scalar(QA_lo[:], tones[:], zA_lo[:], tu3lo[:], mult, add)
                nc.vector.tensor_scalar(QB_lo[:], tones[:], zB_lo[:], tu3lo[:], mult, add)
                nc.vector.tensor_scalar(QA_hi[:], tones[0:CHIA, :], zA_hi[:], tu3hi[:], mult, add)
                nc.vector.tensor_scalar(QB_hi[:], tones[0:CHIA, :], zB_hi[:], tu3hi[:], mult, add)

                # ---- W rows, broadcast across partitions by construction ----
                ps_wA = psw.tile([CLO, L], F32)
                ps_wB = psw.tile([CLO, L], F32)
                nc.tensor.matmul(ps_wA[:], QA_lo[:], phiA_lo[:], start=True, stop=False)
                nc.tensor.matmul(ps_wA[:], QA_hi[:], phiA_hi[:], start=False, stop=True)
                nc.tensor.matmul(ps_wB[:], QB_lo[:], phiB_lo[:], start=True, stop=False)
                nc.tensor.matmul(ps_wB[:], QB_hi[:], phiB_hi[:], start=False, stop=True)

                # ---- out = xp*Wx + yp*Wy ----
                o_lo = outp.tile([CLO, 2, N], F32)
                o_hi = outp.tile([CHI, 2, N], F32)
                t1Al = work.tile([CLO, N], F32, tag="t1l")
                t2Al = work.tile([CLO, N], F32, tag="t2l")
                t1Bl = work.tile([CLO, N], F32, tag="t1l")
                t2Bl = work.tile([CLO, N], F32, tag="t2l")
                t1Ah = work.tile([CHI, N], F32, tag="t1h")
                t2Ah = work.tile([CHI, N], F32, tag="t2h")
                t1Bh = work.tile([CHI, N], F32, tag="t1h")
                t2Bh = work.tile([CHI, N], F32, tag="t2h")
                nc.vector.tensor_mul(t1Al[:], f(xlo[:, 0, :]), ps_wA[:, 0:N])
                nc.vector.tensor_mul(t2Al[:], f(ylo[:, 0, :]), ps_wA[:, N:L])
                nc.vector.tensor_add(o_lo[:, 0, :], t1Al[:], t2Al[:])
                nc.vector.tensor_mul(t1Bl[:], f(xlo[:, 1, :]), ps_wB[:, 0:N])
                nc.vector.tensor_mul(t2Bl[:], f(ylo[:, 1, :]), ps_wB[:, N:L])
                nc.vector.tensor_add(o_lo[:, 1, :], t1Bl[:], t2Bl[:])
                nc.vector.tensor_mul(t1Ah[:], f(xhi[:, 0, :]), ps_wA[0:CHI, 0:N])
                nc.vector.tensor_mul(t2Ah[:], f(yhi[:, 0, :]), ps_wA[0:CHI, N:L])
                nc.vector.tensor_add(o_hi[:, 0, :], t1Ah[:], t2Ah[:])
                nc.vector.tensor_mul(t1Bh[:], f(xhi[:, 1, :]), ps_wB[0:CHI, 0:N])
                nc.vector.tensor_mul(t2Bh[:], f(yhi[:, 1, :]), ps_wB[0:CHI, N:L])
                nc.vector.tensor_add(o_hi[:, 1, :], t1Bh[:], t2Bh[:])

                nc.sync.dma_start(out=outv[0:CLO, b0:b0 + 2, :], in_=o_lo[:])
                nc.sync.dma_start(out=outv[CLO:C, b0:b0 + 2, :], in_=o_hi[:])

    nc.compile()
    return nc


def _host_prepack(d):
    """Fold BN, collapse the linear tail, build per-core constant arrays."""
    f = np.float32
    inv1 = d["g1"] / np.sqrt(d["v1"] + 1e-5)
    W1 = (d["w1"] * inv1[:, None]).astype(f)
    c1 = ((d["b1"] - d["m1"]) * inv1 + d["be1"]).astype(f)
    inv2 = d["g2"] / np.sqrt(d["v2"] + 1e-5)
    W2 = (d["w2"] * inv2[:, None]).astype(f)
    c2 = ((d["b2"] - d["m2"]) * inv2 + d["be2"]).astype(f)

    w4eff = d["w4"][:, :L] + d["w4"][:, L:]
    w5a, w5b = d["w5"][0, :C], d["w5"][0, C:]
    u3 = (w5a @ d["w3"]).astype(f)
    u4 = (w5b @ w4eff).astype(f)
    K = float(w5a @ d["b3"] + w5b @ d["b4"] + d["b5"][0])

    W1T, W2T = np.ascontiguousarray(W1.T), np.ascontiguousarray(W2.T)

    def hi_pad_m(a):  # [k, 64] -> [k, 65] with zero last col
        z = np.zeros((a.shape[0], CHIA), f)
        z[:, :CHI] = a
        return z

    consts = {
        "wxa": np.ascontiguousarray(W1T[:CLO, :CLO]),
        "wxb": hi_pad_m(W1T[:CLO, CLO:C]),
        "wxc": np.ascontiguousarray(W1T[CLO:C, :CLO]),
        "wxd": hi_pad_m(W1T[CLO:C, CLO:C]),
        "wya": np.ascontiguousarray(W2T[:CLO, :CLO]),
        "wyb": hi_pad_m(W2T[:CLO, CLO:C]),
        "wyc": np.ascontiguousarray(W2T[CLO:C, :CLO]),
        "wyd": hi_pad_m(W2T[CLO:C, CLO:C]),
        "c1lo": c1[:CLO, None].copy(),
        "c1hi": np.concatenate([c1[CLO:C], [f(1.0)]])[:, None].astype(f),
        "c2lo": c2[:CLO, None].copy(),
        "c2hi": np.concatenate([c2[CLO:C], [f(1.0)]])[:, None].astype(f),
        "u4lo": np.broadcast_to(u4, (CLO, L)).copy(),
        "u4hi": np.concatenate([np.broadcast_to(u4, (CHI, L)),
                                np.zeros((1, L), f)], axis=0),
        "u3lo": u3[:CLO, None].copy(),
        "u3hi": np.concatenate([u3[CLO:C], [f(K)]])[:, None].astype(f),
        "onesd": np.ones((CLO, CLO), f),
    }
    return {k: np.ascontiguousarray(v, dtype=f) for k, v in consts.items()}


def run(inputs, trace=False):
    d = {k: np.asarray(v) for k, v in inputs.items()}
    consts = _host_prepack(d)

    xyp = np.empty((B, 2, C, N), np.float32)
    xyp[:, 0] = d["x"].transpose(0, 2, 1)
    xyp[:, 1] = d["y"].transpose(0, 2, 1)

    if "nc" not in _CACHE:
        _CACHE["nc"] = _build_program()
    nc = _CACHE["nc"]

    in_maps = []
    for cid in range(NCORES):
        m = dict(consts)
        m["xy"] = np.ascontiguousarray(xyp[cid * NB:(cid + 1) * NB])
        in_maps.append(m)

    res = run_bass_kernel_spmd(nc, in_maps, list(range(NCORES)), trace=trace)
    out = np.concatenate([res.results[i]["out"] for i in range(NCORES)], axis=0)
    return out, res


def kernel(**inputs):
    out, _ = run(inputs, trace=False)
    return out


# revision 12
# speedup vs baseline: 1.2222x; 1.2222x over previous
"""Trainium2 Bass kernel for nn_CRA_46797963657479.

Math: the reference builds per-batch gram matrix A = cat_phi^T cat_phi
([B,392,392]) and feeds concat(A, A^T) through big 1x1 convs.  Since A is
symmetric and everything after cat_phi is linear, the whole tail collapses:

    W[b, l] = (u3 + cat_phi[b] @ u4) . cat_phi[b][:, l] + K
    out[b]  = xp[b] * W[b, :N] + yp[b] * W[b, N:]

with u3 = w5a @ w3, u4 = w5b @ (w4[:, :2N] + w4[:, 2N:]),
K = w5a.b3 + w5b.b4 + b5.  BN folds into the conv weights.  What remains per
batch is two 192x192 matmuls (phi_x, phi_y), a weighted free-dim reduction
(z), one more matmul for W, and an elementwise combine -> memory-bound.

Sharding: pure data parallel, batch 256 -> 32 per core on 8 cores.
"""

import os
import ml_dtypes
import numpy as np

import concourse.bass as bass
import concourse.bacc as bacc
import concourse.tile as tile
from concourse import mybir
from concourse.bass_utils import run_bass_kernel_spmd

F32 = mybir.dt.float32
F32R = mybir.dt.float32r
BF16 = mybir.dt.bfloat16

B, N, C = 256, 196, 192
NCORES = 8
NB = B // NCORES          # 32 batches per core
NPAIR = NB // 2           # 16 pairs per core
L = 2 * N                 # 392 free columns per pair tile / per stream-pack
CLO, CHI = 128, C - 128   # 128 + 64 channel split
CHIA = CHI + 1            # hi chunk augmented with a ones-row (folds +K)

_CACHE = {}


def _build_program():
    nc = bacc.Bacc("TRN2", target_bir_lowering=False, debug=False)

    xy = nc.dram_tensor("xy", [NB, 2, C, N], F32R, kind="ExternalInput")
    out = nc.dram_tensor("out", [NB, C, N], F32, kind="ExternalOutput")

    # weight tiles (lhsT = W^T chunks; m-dim of the hi tiles has a zero
    # 65th column so ACT bias=1 writes a ones-row into phi_hi)
    wxa = nc.dram_tensor("wxa", [CLO, CLO], F32R, kind="ExternalInput")
    wxb = nc.dram_tensor("wxb", [CLO, CHIA], F32R, kind="ExternalInput")
    wxc = nc.dram_tensor("wxc", [CHI, CLO], F32R, kind="ExternalInput")
    wxd = nc.dram_tensor("wxd", [CHI, CHIA], F32R, kind="ExternalInput")
    wya = nc.dram_tensor("wya", [CLO, CLO], F32R, kind="ExternalInput")
    wyb = nc.dram_tensor("wyb", [CLO, CHIA], F32R, kind="ExternalInput")
    wyc = nc.dram_tensor("wyc", [CHI, CLO], F32R, kind="ExternalInput")
    wyd = nc.dram_tensor("wyd", [CHI, CHIA], F32R, kind="ExternalInput")
    c1lo = nc.dram_tensor("c1lo", [CLO, 1], F32, kind="ExternalInput")
    c1hi = nc.dram_tensor("c1hi", [CHIA, 1], F32, kind="ExternalInput")
    c2lo = nc.dram_tensor("c2lo", [CLO, 1], F32, kind="ExternalInput")
    c2hi = nc.dram_tensor("c2hi", [CHIA, 1], F32, kind="ExternalInput")
    u4lo = nc.dram_tensor("u4lo", [CLO, L], F32, kind="ExternalInput")
    u4hi = nc.dram_tensor("u4hi", [CHIA, L], F32, kind="ExternalInput")
    u3lo = nc.dram_tensor("u3lo", [CLO, 1], F32, kind="ExternalInput")
    u3hi = nc.dram_tensor("u3hi", [CHIA, 1], F32, kind="ExternalInput")
    onesd = nc.dram_tensor("onesd", [CLO, CLO], BF16, kind="ExternalInput")

    xyc = xy.rearrange("b s c n -> c b s n")     # [C, NB, 2, N]
    outv = out.rearrange("b c n -> c b n")       # [C, NB, N]

    with tile.TileContext(nc) as tc:
        with (
            tc.tile_pool(name="consts", bufs=1) as consts,
            tc.tile_pool(name="xin", bufs=3) as xin,
            tc.tile_pool(name="phi", bufs=2) as phip,
            tc.tile_pool(name="junk", bufs=2) as junkp,
            tc.tile_pool(name="qp", bufs=3) as qp,
            tc.tile_pool(name="work", bufs=2) as work,
            tc.tile_pool(name="outp", bufs=3) as outp,
            tc.tile_pool(name="psph", bufs=1, space="PSUM") as psph,
            tc.tile_pool(name="psw", bufs=2, space="PSUM") as psw,
        ):
            def cload(dram, shape, dt=F32):
                t = consts.tile(shape, dt, tag=dram.name)
                nc.sync.dma_start(out=t[:], in_=dram[:])
                return t

            twxa = cload(wxa, [CLO, CLO], F32R)
            twxb = cload(wxb, [CLO, CHIA], F32R)
            twxc = cload(wxc, [CHI, CLO], F32R)
            twxd = cload(wxd, [CHI, CHIA], F32R)
            twya = cload(wya, [CLO, CLO], F32R)
            twyb = cload(wyb, [CLO, CHIA], F32R)
            twyc = cload(wyc, [CHI, CLO], F32R)
            twyd = cload(wyd, [CHI, CHIA], F32R)
            tc1lo = cload(c1lo, [CLO, 1])
            tc1hi = cload(c1hi, [CHIA, 1])
            tc2lo = cload(c2lo, [CLO, 1])
            tc2hi = cload(c2hi, [CHIA, 1])
            tu4lo = cload(u4lo, [CLO, L])
            tu4hi = cload(u4hi, [CHIA, L])
            tu3lo = cload(u3lo, [CLO, 1])
            tu3hi = cload(u3hi, [CHIA, 1])
            tones = cload(onesd, [CLO, CLO], BF16)

            def f(ap):
                return ap.bitcast(F32)

            relu = mybir.ActivationFunctionType.Relu
            mult = mybir.AluOpType.mult
            add = mybir.AluOpType.add
            byp = mybir.AluOpType.bypass

            for t in range(NPAIR):
                b0 = 2 * t
                # ---- loads: [channels, stream, batch, N], one DMA per chunk ----
                xin_lo = xin.tile([CLO, 2, 2, N], F32R)
                xin_hi = xin.tile([CHI, 2, 2, N], F32R)
                nc.sync.dma_start(out=xin_lo[:], in_=xyc[0:CLO, b0:b0 + 2, :, :])
                nc.sync.dma_start(out=xin_hi[:], in_=xyc[CLO:C, b0:b0 + 2, :, :])
                xlo2 = xin_lo[:, :, 0, :]
                xhi2 = xin_hi[:, :, 0, :]
                ylo2 = xin_lo[:, :, 1, :]
                yhi2 = xin_hi[:, :, 1, :]

                # ---- phi matmuls (pair-packed, 392 moving cols, f32r) ----
                ps_xlo = psph.tile([CLO, L], F32)
                ps_xhi = psph.tile([CHIA, L], F32)
                ps_ylo = psph.tile([CLO, L], F32)
                ps_yhi = psph.tile([CHIA, L], F32)
                nc.tensor.matmul(ps_xlo[:], twxa[:], xlo2, start=True, stop=False)
                nc.tensor.matmul(ps_xlo[:], twxc[:], xhi2, start=False, stop=True)
                nc.tensor.matmul(ps_xhi[:], twxb[:], xlo2, start=True, stop=False)
                nc.tensor.matmul(ps_xhi[:], twxd[:], xhi2, start=False, stop=True)
                nc.tensor.matmul(ps_ylo[:], twya[:], ylo2, start=True, stop=False)
                nc.tensor.matmul(ps_ylo[:], twyc[:], yhi2, start=False, stop=True)
                nc.tensor.matmul(ps_yhi[:], twyb[:], ylo2, start=True, stop=False)
                nc.tensor.matmul(ps_yhi[:], twyd[:], yhi2, start=False, stop=True)

                # ---- relu+bias -> bf16 phi, repacked per batch [phi_x | phi_y] ----
                phiA_lo = phip.tile([CLO, L], BF16)
                phiB_lo = phip.tile([CLO, L], BF16)
                phiA_hi = phip.tile([CHIA, L], BF16)
                phiB_hi = phip.tile([CHIA, L], BF16)
                nc.scalar.activation(phiA_lo[:, 0:N], ps_xlo[:, 0:N], relu, bias=tc1lo[:])
                nc.scalar.activation(phiA_lo[:, N:L], ps_ylo[:, 0:N], relu, bias=tc2lo[:])
                nc.scalar.activation(phiB_lo[:, 0:N], ps_xlo[:, N:L], relu, bias=tc1lo[:])
                nc.scalar.activation(phiB_lo[:, N:L], ps_ylo[:, N:L], relu, bias=tc2lo[:])
                nc.scalar.activation(phiA_hi[:, 0:N], ps_xhi[:, 0:N], relu, bias=tc1hi[:])
                nc.scalar.activation(phiA_hi[:, N:L], ps_yhi[:, 0:N], relu, bias=tc2hi[:])
                nc.scalar.activation(phiB_hi[:, 0:N], ps_xhi[:, N:L], relu, bias=tc1hi[:])
                nc.scalar.activation(phiB_hi[:, N:L], ps_yhi[:, N:L], relu, bias=tc2hi[:])

                # ---- z = phi @ u4 (fused mult+reduce on DVE, bf16) ----
                jA_lo = junkp.tile([CLO, L], F32, tag="j_lo")
                jB_lo = junkp.tile([CLO, L], F32, tag="j_lo")
                jA_hi = junkp.tile([CHIA, L], F32, tag="j_hi")
                jB_hi = junkp.tile([CHIA, L], F32, tag="j_hi")
                zA_lo = qp.tile([CLO, 1], F32)
                zB_lo = qp.tile([CLO, 1], F32)
                zA_hi = qp.tile([CHIA, 1], F32)
                zB_hi = qp.tile([CHIA, 1], F32)
                nc.vector.scalar_tensor_tensor(
                    out=jA_lo[:], in0=phiA_lo[:], scalar=1.0, in1=tu4lo[:],
                    op0=byp, op1=mult, accum_out=zA_lo[:])
                nc.vector.scalar_tensor_tensor(
                    out=jB_lo[:], in0=phiB_lo[:], scalar=1.0, in1=tu4lo[:],
                    op0=byp, op1=mult, accum_out=zB_lo[:])
                nc.vector.scalar_tensor_tensor(
                    out=jA_hi[:], in0=phiA_hi[:], scalar=1.0, in1=tu4hi[:],
                    op0=byp, op1=mult, accum_out=zA_hi[:])
                nc.vector.scalar_tensor_tensor(
                    out=jB_hi[:], in0=phiB_hi[:], scalar=1.0, in1=tu4hi[:],
                    op0=byp, op1=mult, accum_out=zB_hi[:])

                # ---- Q = ones*z + u3 broadcast along lanes (gpsimd, bf16 out) ----
                QA_lo = qp.tile([CLO, CLO], BF16, tag="Q_lo")
                QB_lo = qp.tile([CLO, CLO], BF16, tag="Q_lo")
                QA_hi = qp.tile([CHIA, CLO], BF16, tag="Q_hi")
                QB_hi = qp.tile([CHIA, CLO], BF16, tag="Q_hi")
                nc.gpsimd.tensor_scalar(QA_lo[:], tones[:], zA_lo[:], tu3lo[:], mult, add)
                nc.gpsimd.tensor_scalar(QB_lo[:], tones[:], zB_lo[:], tu3lo[:], mult, add)
                nc.gpsimd.tensor_scalar(QA_hi[:], tones[0:CHIA, :], zA_hi[:], tu3hi[:], mult, add)
                nc.gpsimd.tensor_scalar(QB_hi[:], tones[0:CHIA, :], zB_hi[:], tu3hi[:], mult, add)

                # ---- W rows (bf16 matmul), broadcast across partitions ----
                ps_wA = psw.tile([CLO, L], F32)
                ps_wB = psw.tile([CLO, L], F32)
                nc.tensor.matmul(ps_wA[:], QA_lo[:], phiA_lo[:], start=True, stop=False)
                nc.tensor.matmul(ps_wA[:], QA_hi[:], phiA_hi[:], start=False, stop=True)
                nc.tensor.matmul(ps_wB[:], QB_lo[:], phiB_lo[:], start=True, stop=False)
                nc.tensor.matmul(ps_wB[:], QB_hi[:], phiB_hi[:], start=False, stop=True)

                # ---- out = xp*Wx + yp*Wy ----
                o_lo = outp.tile([CLO, 2, N], F32)
                o_hi = outp.tile([CHI, 2, N], F32)
                t1_lo = work.tile([CLO, L], F32, tag="t1l")
                t2_lo = work.tile([CLO, L], F32, tag="t2l")
                t1_hi = work.tile([CHI, L], F32, tag="t1h")
                t2_hi = work.tile([CHI, L], F32, tag="t2h")
                nc.vector.tensor_mul(t1_lo[:, 0:N], f(xin_lo[:, 0, 0, :]), ps_wA[:, 0:N])
                nc.vector.tensor_mul(t2_lo[:, 0:N], f(xin_lo[:, 0, 1, :]), ps_wA[:, N:L])
                nc.vector.tensor_mul(t1_lo[:, N:L], f(xin_lo[:, 1, 0, :]), ps_wB[:, 0:N])
                nc.vector.tensor_mul(t2_lo[:, N:L], f(xin_lo[:, 1, 1, :]), ps_wB[:, N:L])
                nc.vector.tensor_add(o_lo[:].rearrange("p a n -> p (a n)"), t1_lo[:], t2_lo[:])
                nc.vector.tensor_mul(t1_hi[:, 0:N], f(xin_hi[:, 0, 0, :]), ps_wA[0:CHI, 0:N])
                nc.vector.tensor_mul(t2_hi[:, 0:N], f(xin_hi[:, 0, 1, :]), ps_wA[0:CHI, N:L])
                nc.vector.tensor_mul(t1_hi[:, N:L], f(xin_hi[:, 1, 0, :]), ps_wB[0:CHI, 0:N])
                nc.vector.tensor_mul(t2_hi[:, N:L], f(xin_hi[:, 1, 1, :]), ps_wB[0:CHI, N:L])
                nc.vector.tensor_add(o_hi[:].rearrange("p a n -> p (a n)"), t1_hi[:], t2_hi[:])

                nc.sync.dma_start(out=outv[0:CLO, b0:b0 + 2, :], in_=o_lo[:])
                nc.sync.dma_start(out=outv[CLO:C, b0:b0 + 2, :], in_=o_hi[:])

    nc.compile()
    return nc


def _host_prepack(d):
    """Fold BN, collapse the linear tail, build per-core constant arrays."""
    f = np.float32
    inv1 = d["g1"] / np.sqrt(d["v1"] + 1e-5)
    W1 = (d["w1"] * inv1[:, None]).astype(f)
    c1 = ((d["b1"] - d["m1"]) * inv1 + d["be1"]).astype(f)
    inv2 = d["g2"] / np.sqrt(d["v2"] + 1e-5)
    W2 = (d["w2"] * inv2[:, None]).astype(f)
    c2 = ((d["b2"] - d["m2"]) * inv2 + d["be2"]).astype(f)

    w4eff = d["w4"][:, :L] + d["w4"][:, L:]
    w5a, w5b = d["w5"][0, :C], d["w5"][0, C:]
    u3 = (w5a @ d["w3"]).astype(f)
    u4 = (w5b @ w4eff).astype(f)
    K = float(w5a @ d["b3"] + w5b @ d["b4"] + d["b5"][0])

    W1T, W2T = np.ascontiguousarray(W1.T), np.ascontiguousarray(W2.T)

    def hi_pad_m(a):  # [k, 64] -> [k, 65] with zero last col
        z = np.zeros((a.shape[0], CHIA), f)
        z[:, :CHI] = a
        return z

    consts = {
        "wxa": np.ascontiguousarray(W1T[:CLO, :CLO]),
        "wxb": hi_pad_m(W1T[:CLO, CLO:C]),
        "wxc": np.ascontiguousarray(W1T[CLO:C, :CLO]),
        "wxd": hi_pad_m(W1T[CLO:C, CLO:C]),
        "wya": np.ascontiguousarray(W2T[:CLO, :CLO]),
        "wyb": hi_pad_m(W2T[:CLO, CLO:C]),
        "wyc": np.ascontiguousarray(W2T[CLO:C, :CLO]),
        "wyd": hi_pad_m(W2T[CLO:C, CLO:C]),
        "c1lo": c1[:CLO, None].copy(),
        "c1hi": np.concatenate([c1[CLO:C], [f(1.0)]])[:, None].astype(f),
        "c2lo": c2[:CLO, None].copy(),
        "c2hi": np.concatenate([c2[CLO:C], [f(1.0)]])[:, None].astype(f),
        "u4lo": np.broadcast_to(u4, (CLO, L)).copy(),
        "u4hi": np.concatenate([np.broadcast_to(u4, (CHI, L)),
                                np.zeros((1, L), f)], axis=0),
        "u3lo": u3[:CLO, None].copy(),
        "u3hi": np.concatenate([u3[CLO:C], [f(K)]])[:, None].astype(f),
        "onesd": np.ones((CLO, CLO), ml_dtypes.bfloat16),
    }
    return {k: np.ascontiguousarray(v) for k, v in consts.items()}


def run(inputs, trace=False):
    d = {k: np.asarray(v) for k, v in inputs.items()}
    consts = _host_prepack(d)

    xyp = np.empty((B, 2, C, N), np.float32)
    xyp[:, 0] = d["x"].transpose(0, 2, 1)
    xyp[:, 1] = d["y"].transpose(0, 2, 1)

    if "nc" not in _CACHE:
        _CACHE["nc"] = _build_program()
    nc = _CACHE["nc"]

    in_maps = []
    for cid in range(NCORES):
        m = dict(consts)
        m["xy"] = np.ascontiguousarray(xyp[cid * NB:(cid + 1) * NB])
        in_maps.append(m)

    res = run_bass_kernel_spmd(nc, in_maps, list(range(NCORES)), trace=trace)
    out = np.concatenate([res.results[i]["out"] for i in range(NCORES)], axis=0)
    return out, res


def kernel(**inputs):
    out, _ = run(inputs, trace=False)
    return out
